# revision 1
# baseline (speedup 1.0000x reference)
"""JoinConvNet Trainium2 kernel — 8-core data-parallel, raw Bass.

Math per subnet (reference):
  conv(x, w)  : x[B,1,L,E], w[C,1,W,E], VALID -> c[B,C,L-W+1]
  m = max_l c ; h = relu(m + b_conv) ; o = relu(h @ w_fc.T + b_fc)
  out[b] = dot(o1[b], o2[b])

Device strategy (per core, 128 batches):
  Host pre-transposes x to X^T[E, B*L] and packs into 5 row-planes of <=128:
    plane0/1 = x1 e[0:128>, e[128:256>; plane2/3 = x2; plane4 = x1 e[256:300>
    at partitions 0:44, x2 e[256:300> at partitions 44:88, zeros elsewhere.
  Conv = 9 accumulating matmuls per 400-position chunk (3 taps x 3 e-chunks),
  tap shift folded into the rhs free-dim offset. PSUM [80,400] -> per-batch
  reduce_max -> H[80,128] -> bias+relu -> FC matmul -> bias+relu -> elementwise
  mul -> ones-matmul partition sum -> [1,128] out.

Conv matmuls run in float32r (fp32 bytes, full PE rate at N>=256).
"""
import os
import numpy as np
from contextlib import ExitStack

import concourse.bass as bass
import concourse.mybir as mybir
from concourse.bass_utils import run_bass_kernel_spmd

B, L, E = 1024, 200, 300
C, W, O = 80, 3, 30
NCORES = 8
BS = B // NCORES            # 128 batches/core
POS = BS * L                # 25600 positions/core
PADPOS = POS + 2
NSLAB = int(os.environ.get("K_NSLAB", "32"))
SLAB = POS // NSLAB         # positions per slab
SLABW = SLAB + 2            # loaded columns per slab
NG = SLAB // 400            # 400-position groups per slab
SLOTS = int(os.environ.get("K_SLOTS", "4"))
SPLIT_DMA = bool(int(os.environ.get("K_SPLIT_DMA", "0")))
GN = 400                    # matmul moving size
VALID = L - W + 1           # 198

CONV_DT = mybir.dt.float32r
F32 = mybir.dt.float32

LAST_RESULT = None
TRACE = bool(os.environ.get("KERNEL_TRACE"))
_NC_CACHE = {}


def _build_nc():
    nc = bass.Bass()
    xp = nc.declare_dram_parameter("xp", [5, 128, PADPOS], CONV_DT, isOutput=False)
    wst = nc.declare_dram_parameter("wst", [128, 18 * C], CONV_DT, isOutput=False)
    wf = nc.declare_dram_parameter("wf", [C, 2 * O], F32, isOutput=False)
    bc = nc.declare_dram_parameter("bc", [C, 2], F32, isOutput=False)
    bf = nc.declare_dram_parameter("bf", [O, 2], F32, isOutput=False)
    out = nc.declare_dram_parameter("out", [1, BS], F32, isOutput=True)

    with ExitStack() as ctx:
        X = ctx.enter_context(nc.sbuf_tensor([128, SLOTS, 5, SLABW], CONV_DT))
        Wc = ctx.enter_context(nc.sbuf_tensor([128, 18 * C], CONV_DT))
        Wf = ctx.enter_context(nc.sbuf_tensor([C, 2 * O], F32))
        Bc = ctx.enter_context(nc.sbuf_tensor([C, 2], F32))
        Bf = ctx.enter_context(nc.sbuf_tensor([O, 2], F32))
        ones = ctx.enter_context(nc.sbuf_tensor([O, 1], F32))
        H = ctx.enter_context(nc.sbuf_tensor([C, 2, BS], F32))
        Hr = ctx.enter_context(nc.sbuf_tensor([C, 2, BS], F32))
        Ofc = ctx.enter_context(nc.sbuf_tensor([O, 2, BS], F32))
        P = ctx.enter_context(nc.sbuf_tensor([O, BS], F32))
        osb = ctx.enter_context(nc.sbuf_tensor([1, BS], F32))
        cps = [ctx.enter_context(nc.psum_tensor(f"cps{i}", [C, GN], F32)) for i in range(4)]
        fps = [ctx.enter_context(nc.psum_tensor(f"fps{i}", [O, BS], F32)) for i in range(2)]
        dps = ctx.enter_context(nc.psum_tensor([1, BS], F32))

        dma_sem = ctx.enter_context(nc.semaphore("dma_sem"))
        pe_sem = ctx.enter_context(nc.semaphore("pe_sem"))
        red_sem = ctx.enter_context(nc.semaphore("red_sem"))
        act_sem = ctx.enter_context(nc.semaphore("act_sem"))
        fc_sem = ctx.enter_context(nc.semaphore("fc_sem"))
        block = ctx.enter_context(nc.Block())

        @block.sync
        def _(sync):
            sync.dma_start(out=Wc[:, :], in_=wst[:, :]).then_inc(dma_sem, 16)
            sync.dma_start(out=Wf[:, :], in_=wf[:, :]).then_inc(dma_sem, 16)
            sync.dma_start(out=Bc[:, :], in_=bc[:, :]).then_inc(dma_sem, 16)
            sync.dma_start(out=Bf[:, :], in_=bf[:, :]).then_inc(dma_sem, 16)
            for s in range(NSLAB):
                if s >= SLOTS:
                    # slot s%SLOTS free once PE finished slab s-SLOTS
                    sync.wait_ge(pe_sem, 2 * NG * (s - SLOTS + 1))
                for j in range(5):
                    if SPLIT_DMA and j >= 2:
                        continue
                    sync.dma_start(
                        out=X[:, s % SLOTS, j, :],
                        in_=xp[j, :, s * SLAB : s * SLAB + SLABW],
                    ).then_inc(dma_sem, 16)
            sync.wait_ge(act_sem, 3)
            sync.dma_start(out=out[:, :], in_=osb[:, :]).then_inc(dma_sem, 16)

        @block.tensor
        def _(tensor):
            k = 0
            for s in range(NSLAB):
                tensor.wait_ge(dma_sem, 64 + 5 * 16 * (s + 1))
                for g in range(NG):
                    for n in range(2):
                        if k >= 4:
                            tensor.wait_ge(red_sem, k - 3)
                        idx = 0
                        for w in range(W):
                            for j in range(3):
                                plane = (2 * n + j) if j < 2 else 4
                                col = (n * 9 + w * 3 + j) * C
                                mm = tensor.matmul(
                                    cps[k % 4][:, :],
                                    Wc[:, col : col + C],
                                    X[:, s % SLOTS, plane, g * GN + w : g * GN + w + GN],
                                    start=(idx == 0),
                                    stop=(idx == 8),
                                )
                                idx += 1
                        mm.then_inc(pe_sem, 1)
                        k += 1
            # FC + dot tail
            tensor.wait_ge(act_sem, 1)
            tensor.matmul(fps[0][:, :], Wf[:, 0:O], Hr[:, 0, :], start=True, stop=True)
            tensor.matmul(
                fps[1][:, :], Wf[:, O : 2 * O], Hr[:, 1, :], start=True, stop=True
            ).then_inc(fc_sem, 1)
            tensor.wait_ge(red_sem, 2 * NG * NSLAB + 1)
            tensor.matmul(dps[:, :], ones[:, :], P[:, :], start=True, stop=True).then_inc(
                fc_sem, 1
            )

        @block.vector
        def _(vector):
            vector.memset(ones[:, :], 1.0)
            k = 0
            for s in range(NSLAB):
                for g in range(NG):
                    p = s * NG + g  # batch pair index
                    for n in range(2):
                        vector.wait_ge(pe_sem, k + 1)
                        vector.reduce_max(
                            H[:, n, 2 * p : 2 * p + 1],
                            cps[k % 4][:, 0:VALID],
                            axis=mybir.AxisListType.X,
                        )
                        vector.reduce_max(
                            H[:, n, 2 * p + 1 : 2 * p + 2],
                            cps[k % 4][:, 200 : 200 + VALID],
                            axis=mybir.AxisListType.X,
                        ).then_inc(red_sem, 1)
                        k += 1
            vector.wait_ge(act_sem, 2)
            vector.tensor_mul(P[:, :], Ofc[:, 0, :], Ofc[:, 1, :]).then_inc(red_sem, 1)

        @block.scalar
        def _(scalar):
            if SPLIT_DMA:
                for s in range(NSLAB):
                    if s >= SLOTS:
                        scalar.wait_ge(pe_sem, 2 * NG * (s - SLOTS + 1))
                    for j in range(2, 5):
                        scalar.dma_start(
                            out=X[:, s % SLOTS, j, :],
                            in_=xp[j, :, s * SLAB : s * SLAB + SLABW],
                        ).then_inc(dma_sem, 16)
            scalar.wait_ge(red_sem, 2 * NG * NSLAB)
            scalar.activation(
                Hr[:, 0, :], H[:, 0, :], mybir.ActivationFunctionType.Relu,
                bias=Bc[:, 0:1],
            )
            scalar.activation(
                Hr[:, 1, :], H[:, 1, :], mybir.ActivationFunctionType.Relu,
                bias=Bc[:, 1:2],
            ).then_inc(act_sem, 1)
            scalar.wait_ge(fc_sem, 1)
            scalar.activation(
                Ofc[:, 0, :], fps[0][:, :], mybir.ActivationFunctionType.Relu,
                bias=Bf[:, 0:1],
            )
            scalar.activation(
                Ofc[:, 1, :], fps[1][:, :], mybir.ActivationFunctionType.Relu,
                bias=Bf[:, 1:2],
            ).then_inc(act_sem, 1)
            scalar.wait_ge(fc_sem, 2)
            scalar.copy(osb[:, :], dps[:, :]).then_inc(act_sem, 1)

    return nc


def _prep_weights(w_conv1, w_conv2, w_fc1, w_fc2, b_conv1, b_conv2, b_fc1, b_fc2):
    wst = np.zeros((128, 18, C), dtype=np.float32)
    for n, wc in enumerate((w_conv1, w_conv2)):
        wcs = wc[:, 0]  # [C, W, E]
        for w in range(W):
            for j in range(3):
                idx = n * 9 + w * 3 + j
                if j < 2:
                    wst[:, idx, :] = wcs[:, w, 128 * j : 128 * (j + 1)].T
                else:
                    sl = wcs[:, w, 256:300].T  # [44, C]
                    if n == 0:
                        wst[0:44, idx, :] = sl
                    else:
                        wst[44:88, idx, :] = sl
    wf = np.concatenate([w_fc1.T, w_fc2.T], axis=1).astype(np.float32)  # [C, 2O]
    bc = np.stack([b_conv1, b_conv2], axis=1).astype(np.float32)  # [C, 2]
    bf = np.stack([b_fc1, b_fc2], axis=1).astype(np.float32)  # [O, 2]
    return wst.reshape(128, 18 * C), wf, bc, bf


def kernel(x1, x2, w_conv1, b_conv1, w_fc1, b_fc1, w_conv2, b_conv2, w_fc2, b_fc2):
    global LAST_RESULT
    x1 = np.ascontiguousarray(np.asarray(x1, dtype=np.float32))
    x2 = np.ascontiguousarray(np.asarray(x2, dtype=np.float32))
    wst, wf, bc, bf = _prep_weights(
        np.asarray(w_conv1, np.float32), np.asarray(w_conv2, np.float32),
        np.asarray(w_fc1, np.float32), np.asarray(w_fc2, np.float32),
        np.asarray(b_conv1, np.float32), np.asarray(b_conv2, np.float32),
        np.asarray(b_fc1, np.float32), np.asarray(b_fc2, np.float32),
    )

    if "nc" not in _NC_CACHE:
        _NC_CACHE["nc"] = _build_nc()
    nc = _NC_CACHE["nc"]

    in_maps = []
    for c in range(NCORES):
        xs1 = x1[c * BS : (c + 1) * BS, 0].reshape(POS, E).T  # [300, POS]
        xs2 = x2[c * BS : (c + 1) * BS, 0].reshape(POS, E).T
        xp = np.zeros((5, 128, PADPOS), dtype=np.float32)
        xp[0, :, :POS] = xs1[0:128]
        xp[1, :, :POS] = xs1[128:256]
        xp[2, :, :POS] = xs2[0:128]
        xp[3, :, :POS] = xs2[128:256]
        xp[4, 0:44, :POS] = xs1[256:300]
        xp[4, 44:88, :POS] = xs2[256:300]
        in_maps.append({"xp": xp, "wst": wst, "wf": wf, "bc": bc, "bf": bf})

    res = run_bass_kernel_spmd(nc, in_maps, list(range(NCORES)), trace=TRACE)
    LAST_RESULT = res
    return np.concatenate(
        [res.results[c]["out"].reshape(BS, 1) for c in range(NCORES)], axis=0
    )



# revision 20
# speedup vs baseline: 1.1322x; 1.1322x over previous
"""JoinConvNet Trainium2 kernel — 8-core data-parallel, raw Bass.

Math per subnet (reference):
  conv(x, w)  : x[B,1,L,E], w[C,1,W,E], VALID -> c[B,C,L-W+1]
  m = max_l c ; h = relu(m + b_conv) ; o = relu(h @ w_fc.T + b_fc)
  out[b] = dot(o1[b], o2[b])

Device strategy (per core, 128 batches):
  Host pre-transposes x to X^T[E, B*L] (fp16) and packs:
    xp1[4] = full 128-row e-chunks: x1 e[0:128>, e[128:256>, x2 same.
    xp2[3] = 88-row leftover planes with tap shifts pre-applied on host:
      a = [x1 e[256:300> @p ; x1 @p+1], b = [x1 @p+2 ; x2 @p],
      c = [x2 @p+1 ; x2 @p+2].
  Per (group of 400 positions, subnet): 8 accumulating fp16 matmuls
  (K=128 x 6 full-chunk taps via rhs offset + 2 merged 88-row leftover
  matmuls) — the ceil(900/128)=8 contraction floor. PSUM [80,400] ->
  per-batch reduce_max -> H[80,128] -> bias+relu -> FC matmul ->
  bias+relu -> elementwise mul -> ones-matmul partition sum -> [1,128].
"""
import os
import numpy as np
from contextlib import ExitStack

import concourse.bass as bass
import concourse.mybir as mybir
from concourse.bass_utils import run_bass_kernel_spmd

B, L, E = 1024, 200, 300
C, W, O = 80, 3, 30
NCORES = 8
BS = B // NCORES            # 128 batches/core
POS = BS * L                # 25600 positions/core
PADPOS = POS + 2
NSLAB = int(os.environ.get("K_NSLAB", "32"))
SLAB = POS // NSLAB         # positions per slab
SLABW = SLAB + 2            # loaded columns per slab
NG = SLAB // 400            # 400-position groups per slab
SLOTS = int(os.environ.get("K_SLOTS", "4"))
GN = 400                    # matmul moving size
VALID = L - W + 1           # 198

F16 = mybir.dt.float16
F32 = mybir.dt.float32

LAST_RESULT = None
TRACE = bool(os.environ.get("KERNEL_TRACE"))
_NC_CACHE = {}

# stationary block order within a chain (per subnet): j0w0..j0w2, j1w0..j1w2,
# then two merged leftover blocks. 16 blocks total: idx = n*8 + pos.
NBLK = 16


def _build_nc():
    nc = bass.Bass()
    xp1 = nc.declare_dram_parameter("xp1", [128, 4, PADPOS], F16, isOutput=False)
    xp2 = nc.declare_dram_parameter("xp2", [88, 3, PADPOS], F16, isOutput=False)
    wst = nc.declare_dram_parameter("wst", [128, NBLK * C], F16, isOutput=False)
    wf = nc.declare_dram_parameter("wf", [C, 2 * O], F32, isOutput=False)
    bc = nc.declare_dram_parameter("bc", [C, 2], F32, isOutput=False)
    bf = nc.declare_dram_parameter("bf", [O, 2], F32, isOutput=False)
    out = nc.declare_dram_parameter("out", [1, BS], F32, isOutput=True)

    with ExitStack() as ctx:
        X = ctx.enter_context(nc.sbuf_tensor([128, SLOTS, 4, SLABW], F16))
        X2 = ctx.enter_context(nc.sbuf_tensor([88, SLOTS, 3, SLABW], F16))
        Wc = ctx.enter_context(nc.sbuf_tensor([128, NBLK * C], F16))
        Wf = ctx.enter_context(nc.sbuf_tensor([C, 2 * O], F32))
        Bc = ctx.enter_context(nc.sbuf_tensor([C, 2], F32))
        Bf = ctx.enter_context(nc.sbuf_tensor([O, 2], F32))
        ones = ctx.enter_context(nc.sbuf_tensor([O, 1], F32))
        H = ctx.enter_context(nc.sbuf_tensor([C, 2, BS], F32))
        Hr = ctx.enter_context(nc.sbuf_tensor([C, 2, BS], F32))
        Ofc = ctx.enter_context(nc.sbuf_tensor([O, 2, BS], F32))
        P = ctx.enter_context(nc.sbuf_tensor([O, BS], F32))
        osb = ctx.enter_context(nc.sbuf_tensor([1, BS], F32))
        cps = [ctx.enter_context(nc.psum_tensor(f"cps{i}", [C, GN], F32)) for i in range(4)]
        fps = [ctx.enter_context(nc.psum_tensor(f"fps{i}", [O, BS], F32)) for i in range(2)]
        dps = ctx.enter_context(nc.psum_tensor([1, BS], F32))

        dma_sem = ctx.enter_context(nc.semaphore("dma_sem"))
        pe_sem = ctx.enter_context(nc.semaphore("pe_sem"))
        red_sem = ctx.enter_context(nc.semaphore("red_sem"))
        act_sem = ctx.enter_context(nc.semaphore("act_sem"))
        fc_sem = ctx.enter_context(nc.semaphore("fc_sem"))
        block = ctx.enter_context(nc.Block())

        def slab_ready(s):
            # dma_sem after slab s fully loaded (wst first, tail weights
            # interleaved after slab 0); every DMA incs by 16, 7 per slab
            return 16 + 112 * (s + 1) + (48 if s >= 1 else 0)

        @block.sync
        def _(sync):
            sync.dma_start(out=Wc[:, :], in_=wst[:, :]).then_inc(dma_sem, 16)
            for s in range(NSLAB):
                if s == 1:
                    sync.dma_start(out=Wf[:, :], in_=wf[:, :]).then_inc(dma_sem, 16)
                    sync.dma_start(out=Bc[:, :], in_=bc[:, :]).then_inc(dma_sem, 16)
                    sync.dma_start(out=Bf[:, :], in_=bf[:, :]).then_inc(dma_sem, 16)
                if s >= SLOTS:
                    # slot s%SLOTS free once PE finished slab s-SLOTS
                    sync.wait_ge(pe_sem, 2 * NG * (s - SLOTS + 1))
                for j in range(4):
                    sync.dma_start(
                        out=X[:, s % SLOTS, j, :],
                        in_=xp1[:, j, s * SLAB : s * SLAB + SLABW],
                    ).then_inc(dma_sem, 16)
                for j in range(3):
                    sync.dma_start(
                        out=X2[:, s % SLOTS, j, :],
                        in_=xp2[:, j, s * SLAB : s * SLAB + SLABW],
                    ).then_inc(dma_sem, 16)
            sync.wait_ge(act_sem, 3)
            sync.dma_start(out=out[:, :], in_=osb[:, :]).then_inc(dma_sem, 16)

        @block.tensor
        def _(tensor):
            k = 0
            for s in range(NSLAB):
                tensor.wait_ge(dma_sem, slab_ready(s))
                for g in range(NG):
                    for n in range(2):
                        if k >= 4:
                            tensor.wait_ge(red_sem, k - 3)
                        # 6 full-chunk matmuls: tap shift via rhs offset
                        idx = 0
                        for j in range(2):
                            plane = 2 * n + j
                            for w in range(W):
                                col = (n * 8 + j * 3 + w) * C
                                tensor.matmul(
                                    cps[k % 4][:, :],
                                    Wc[:, col : col + C],
                                    X[:, s % SLOTS, plane, g * GN + w : g * GN + w + GN],
                                    start=(idx == 0),
                                    stop=False,
                                )
                                idx += 1
                        # 2 merged 88-row leftover matmuls (shifts pre-applied)
                        for i in range(2):
                            pl = n + i  # subnet0: planes a,b ; subnet1: b,c
                            col = (n * 8 + 6 + i) * C
                            mm = tensor.matmul(
                                cps[k % 4][:, :],
                                Wc[0:88, col : col + C],
                                X2[:, s % SLOTS, pl, g * GN : g * GN + GN],
                                start=False,
                                stop=(i == 1),
                            )
                        mm.then_inc(pe_sem, 1)
                        k += 1
            # FC + dot tail
            tensor.wait_ge(act_sem, 1)
            tensor.matmul(fps[0][:, :], Wf[:, 0:O], Hr[:, 0, :], start=True, stop=True)
            tensor.matmul(
                fps[1][:, :], Wf[:, O : 2 * O], Hr[:, 1, :], start=True, stop=True
            ).then_inc(fc_sem, 1)
            tensor.wait_ge(red_sem, 2 * NG * NSLAB + 1)
            tensor.matmul(dps[:, :], ones[:, :], P[:, :], start=True, stop=True).then_inc(
                fc_sem, 1
            )

        @block.vector
        def _(vector):
            vector.memset(ones[:, :], 1.0)
            k = 0
            for s in range(NSLAB):
                for g in range(NG):
                    p = s * NG + g  # batch pair index
                    for n in range(2):
                        vector.wait_ge(pe_sem, k + 1)
                        vector.reduce_max(
                            H[:, n, 2 * p : 2 * p + 1],
                            cps[k % 4][:, 0:VALID],
                            axis=mybir.AxisListType.X,
                        )
                        vector.reduce_max(
                            H[:, n, 2 * p + 1 : 2 * p + 2],
                            cps[k % 4][:, 200 : 200 + VALID],
                            axis=mybir.AxisListType.X,
                        ).then_inc(red_sem, 1)
                        k += 1
            vector.wait_ge(act_sem, 2)
            vector.tensor_mul(P[:, :], Ofc[:, 0, :], Ofc[:, 1, :]).then_inc(red_sem, 1)

        @block.scalar
        def _(scalar):
            scalar.wait_ge(red_sem, 2 * NG * NSLAB)
            scalar.wait_ge(dma_sem, 176)
            scalar.activation(
                Hr[:, 0, :], H[:, 0, :], mybir.ActivationFunctionType.Relu,
                bias=Bc[:, 0:1],
            )
            scalar.activation(
                Hr[:, 1, :], H[:, 1, :], mybir.ActivationFunctionType.Relu,
                bias=Bc[:, 1:2],
            ).then_inc(act_sem, 1)
            scalar.wait_ge(fc_sem, 1)
            scalar.activation(
                Ofc[:, 0, :], fps[0][:, :], mybir.ActivationFunctionType.Relu,
                bias=Bf[:, 0:1],
            )
            scalar.activation(
                Ofc[:, 1, :], fps[1][:, :], mybir.ActivationFunctionType.Relu,
                bias=Bf[:, 1:2],
            ).then_inc(act_sem, 1)
            scalar.wait_ge(fc_sem, 2)
            scalar.copy(osb[:, :], dps[:, :]).then_inc(act_sem, 1)

    return nc


def _prep_weights(w_conv1, w_conv2, w_fc1, w_fc2, b_conv1, b_conv2, b_fc1, b_fc2):
    wst = np.zeros((128, NBLK, C), dtype=np.float32)
    for n, wc in enumerate((w_conv1, w_conv2)):
        wcs = wc[:, 0]  # [C, W, E]
        for j in range(2):
            for w in range(W):
                wst[:, n * 8 + j * 3 + w, :] = wcs[:, w, 128 * j : 128 * (j + 1)].T
        lo = wcs[:, :, 256:300]  # [C, W, 44]
        if n == 0:
            # blk6: plane a = [x1@p (w0) ; x1@p+1 (w1)]
            wst[0:44, 6, :] = lo[:, 0, :].T
            wst[44:88, 6, :] = lo[:, 1, :].T
            # blk7: plane b rows 0:44 = x1@p+2 (w2)
            wst[0:44, 7, :] = lo[:, 2, :].T
        else:
            # blk14: plane b rows 44:88 = x2@p (w0)
            wst[44:88, 14, :] = lo[:, 0, :].T
            # blk15: plane c = [x2@p+1 (w1) ; x2@p+2 (w2)]
            wst[0:44, 15, :] = lo[:, 1, :].T
            wst[44:88, 15, :] = lo[:, 2, :].T
    wf = np.concatenate([w_fc1.T, w_fc2.T], axis=1).astype(np.float32)  # [C, 2O]
    bc = np.stack([b_conv1, b_conv2], axis=1).astype(np.float32)  # [C, 2]
    bf = np.stack([b_fc1, b_fc2], axis=1).astype(np.float32)  # [O, 2]
    return wst.reshape(128, NBLK * C).astype(np.float16), wf, bc, bf


def kernel(x1, x2, w_conv1, b_conv1, w_fc1, b_fc1, w_conv2, b_conv2, w_fc2, b_fc2):
    global LAST_RESULT
    x1 = np.asarray(x1, dtype=np.float32)
    x2 = np.asarray(x2, dtype=np.float32)
    wst, wf, bc, bf = _prep_weights(
        np.asarray(w_conv1, np.float32), np.asarray(w_conv2, np.float32),
        np.asarray(w_fc1, np.float32), np.asarray(w_fc2, np.float32),
        np.asarray(b_conv1, np.float32), np.asarray(b_conv2, np.float32),
        np.asarray(b_fc1, np.float32), np.asarray(b_fc2, np.float32),
    )

    if "nc" not in _NC_CACHE:
        _NC_CACHE["nc"] = _build_nc()
    nc = _NC_CACHE["nc"]

    in_maps = []
    for c in range(NCORES):
        xs1 = x1[c * BS : (c + 1) * BS, 0].reshape(POS, E).T.astype(np.float16)
        xs2 = x2[c * BS : (c + 1) * BS, 0].reshape(POS, E).T.astype(np.float16)
        xp1 = np.zeros((128, 4, PADPOS), dtype=np.float16)
        xp1[:, 0, :POS] = xs1[0:128]
        xp1[:, 1, :POS] = xs1[128:256]
        xp1[:, 2, :POS] = xs2[0:128]
        xp1[:, 3, :POS] = xs2[128:256]
        # leftover planes with pre-applied tap shifts
        pad1 = np.zeros((44, POS + 4), dtype=np.float16)
        pad2 = np.zeros((44, POS + 4), dtype=np.float16)
        pad1[:, :POS] = xs1[256:300]
        pad2[:, :POS] = xs2[256:300]
        xp2 = np.zeros((88, 3, PADPOS), dtype=np.float16)
        xp2[0:44, 0] = pad1[:, 0:PADPOS]         # x1 @p   (w0)
        xp2[44:88, 0] = pad1[:, 1 : 1 + PADPOS]  # x1 @p+1 (w1)
        xp2[0:44, 1] = pad1[:, 2 : 2 + PADPOS]   # x1 @p+2 (w2)
        xp2[44:88, 1] = pad2[:, 0:PADPOS]        # x2 @p   (w0)
        xp2[0:44, 2] = pad2[:, 1 : 1 + PADPOS]   # x2 @p+1 (w1)
        xp2[44:88, 2] = pad2[:, 2 : 2 + PADPOS]  # x2 @p+2 (w2)
        in_maps.append(
            {"xp1": xp1, "xp2": xp2, "wst": wst, "wf": wf, "bc": bc, "bf": bf}
        )

    res = run_bass_kernel_spmd(nc, in_maps, list(range(NCORES)), trace=TRACE)
    LAST_RESULT = res
    return np.concatenate(
        [res.results[c]["out"].reshape(BS, 1) for c in range(NCORES)], axis=0
    )


# revision 36
# speedup vs baseline: 1.9598x; 1.7310x over previous
"""JoinConvNet Trainium2 kernel — 8-core data-parallel, raw Bass.

Math per subnet (reference):
  conv(x, w)  : x[B,1,L,E], w[C,1,W,E], VALID -> c[B,C,L-W+1]
  m = max_l c ; h = relu(m + b_conv) ; o = relu(h @ w_fc.T + b_fc)
  out[b] = dot(o1[b], o2[b])

Device strategy (per core, 128 batches):
  The e[0:256) part of the contraction runs in fp8e4m3 with
  MatmulPerfMode.DoubleRow: host packs pairs (x[e], x[e+128]) interleaved
  along the moving free dim and weights as [K,2,M] pair tiles, so one
  DoubleRow matmul per tap contracts 256 e-values at 0.5 cycles/row.
  The e[256:300) leftover runs in fp16 via the shared 88-row plane
  (x1 rows 0:44, x2 rows 44:88, zero-padded stationary) with the tap
  shift applied through the moving-column offset.

  Per (group of 400 positions, subnet): 3 DoubleRow + 3 fp16 accumulating
  matmuls -> PSUM [80,400] -> per-batch reduce_max -> H[80,128] ->
  bias+relu -> FC matmul -> bias+relu -> elementwise mul -> ones-matmul
  partition sum -> [1,128] out.

  Startup is bootstrapped with small leading DMAs so the PE starts early.
  fp8 end-to-end rel err vs the fp32 reference: ~1.5e-2 (measured, fixed
  inputs), under the 2e-2 gate.
"""
import os
import numpy as np
import ml_dtypes
from contextlib import ExitStack

import concourse.bass as bass
import concourse.mybir as mybir
from concourse.bass_utils import run_bass_kernel_spmd

B, L, E = 1024, 200, 300
C, W, O = 80, 3, 30
NCORES = 8
BS = B // NCORES            # 128 batches/core
POS = BS * L                # 25600 positions/core
PADPOS = POS + 2
NSLAB = int(os.environ.get("K_NSLAB", "32"))
SLAB = POS // NSLAB         # positions per slab
SLABW = SLAB + 2            # loaded columns per slab
NG = SLAB // 400            # 400-position groups per slab
SLOTS = int(os.environ.get("K_SLOTS", "4"))
GN = 400                    # matmul moving size
VALID = L - W + 1           # 198

F8 = mybir.dt.float8e4
F16 = mybir.dt.float16
F32 = mybir.dt.float32
NP_F8 = ml_dtypes.float8_e4m3

LAST_RESULT = None
TRACE = bool(os.environ.get("KERNEL_TRACE"))
_NC_CACHE = {}


SLABW8 = (SLABW + 15) // 16 * 16  # pair-plane stride must be 16B-aligned
WSCALE = 64.0  # lift conv weights (~0.02) out of the e4m3 subnormal range


def _build_nc():
    nc = bass.Bass()
    xq8 = nc.declare_dram_parameter("xq8", [128, 2, 2, PADPOS], F8, isOutput=False)
    xp2 = nc.declare_dram_parameter("xp2", [88, PADPOS], F16, isOutput=False)
    wst8 = nc.declare_dram_parameter("wst8", [128, 6, 2, C], F8, isOutput=False)
    wst16 = nc.declare_dram_parameter("wst16", [88, 6 * C], F16, isOutput=False)
    wf = nc.declare_dram_parameter("wf", [C, 2 * O], F32, isOutput=False)
    bc = nc.declare_dram_parameter("bc", [C, 2], F32, isOutput=False)
    bf = nc.declare_dram_parameter("bf", [O, 2], F32, isOutput=False)
    out = nc.declare_dram_parameter("out", [1, BS], F32, isOutput=True)

    with ExitStack() as ctx:
        X8 = ctx.enter_context(nc.sbuf_tensor([128, SLOTS, 2, 2, SLABW8], F8))
        X2 = ctx.enter_context(nc.sbuf_tensor([88, SLOTS, SLABW], F16))
        Wc8 = ctx.enter_context(nc.sbuf_tensor([128, 6, 2, C], F8))
        Wc16 = ctx.enter_context(nc.sbuf_tensor([88, 6 * C], F16))
        Wf = ctx.enter_context(nc.sbuf_tensor([C, 2 * O], F32))
        Bc = ctx.enter_context(nc.sbuf_tensor([C, 2], F32))
        Bf = ctx.enter_context(nc.sbuf_tensor([O, 2], F32))
        ones = ctx.enter_context(nc.sbuf_tensor([O, 1], F32))
        H = ctx.enter_context(nc.sbuf_tensor([C, 2, BS], F32))
        Hr = ctx.enter_context(nc.sbuf_tensor([C, 2, BS], F32))
        Ofc = ctx.enter_context(nc.sbuf_tensor([O, 2, BS], F32))
        P = ctx.enter_context(nc.sbuf_tensor([O, BS], F32))
        osb = ctx.enter_context(nc.sbuf_tensor([1, BS], F32))
        cps = [ctx.enter_context(nc.psum_tensor(f"cps{i}", [C, GN], F32)) for i in range(4)]
        fps = [ctx.enter_context(nc.psum_tensor(f"fps{i}", [O, BS], F32)) for i in range(2)]
        dps = ctx.enter_context(nc.psum_tensor([1, BS], F32))

        dma_sem = ctx.enter_context(nc.semaphore("dma_sem"))
        dma2_sem = ctx.enter_context(nc.semaphore("dma2_sem"))
        pe_sem = ctx.enter_context(nc.semaphore("pe_sem"))
        red_sem = ctx.enter_context(nc.semaphore("red_sem"))
        act_sem = ctx.enter_context(nc.semaphore("act_sem"))
        fc_sem = ctx.enter_context(nc.semaphore("fc_sem"))
        block = ctx.enter_context(nc.Block())

        # sync queue: 4 boot DMAs (64), X8 slab s at 64+16(s+1); wf/bc/bf
        # (+48) interleaved after slab 2.
        def slab_ready(s):
            return 64 + 16 * (s + 1) + (48 if s >= 3 else 0)

        # scalar queue: 2 boot DMAs (32), X2 slab s at 32+16(s+1)
        def slab2_ready(s):
            return 32 + 16 * (s + 1)

        @block.sync
        def _(sync):
            # bootstrap: first-group slices so the PE starts early
            sync.dma_start(out=X8[:, 0, 0, :, 0:402], in_=xq8[:, 0, :, 0:402]).then_inc(dma_sem, 16)
            sync.dma_start(out=Wc8[:, 0:3, :, :], in_=wst8[:, 0:3, :, :]).then_inc(dma_sem, 16)
            sync.dma_start(out=X8[:, 0, 1, :, 0:402], in_=xq8[:, 1, :, 0:402]).then_inc(dma_sem, 16)
            sync.dma_start(out=Wc8[:, 3:6, :, :], in_=wst8[:, 3:6, :, :]).then_inc(dma_sem, 16)
            for s in range(NSLAB):
                if s == 3:
                    sync.dma_start(out=Wf[:, :], in_=wf[:, :]).then_inc(dma_sem, 16)
                    sync.dma_start(out=Bc[:, :], in_=bc[:, :]).then_inc(dma_sem, 16)
                    sync.dma_start(out=Bf[:, :], in_=bf[:, :]).then_inc(dma_sem, 16)
                if s >= SLOTS:
                    # slot s%SLOTS free once PE finished slab s-SLOTS
                    sync.wait_ge(pe_sem, 2 * NG * (s - SLOTS + 1))
                sync.dma_start(
                    out=X8[:, s % SLOTS, :, :, 0:SLABW],
                    in_=xq8[:, :, :, s * SLAB : s * SLAB + SLABW],
                ).then_inc(dma_sem, 16)
            sync.wait_ge(act_sem, 3)
            sync.dma_start(out=out[:, :], in_=osb[:, :]).then_inc(dma_sem, 16)

        @block.tensor
        def _(tensor):
            k = 0
            for s in range(NSLAB):
                if s >= 1:
                    tensor.wait_ge(dma_sem, slab_ready(s))
                    tensor.wait_ge(dma2_sem, slab2_ready(s))
                for g in range(NG):
                    for n in range(2):
                        if s == 0 and g == 0 and n == 0:
                            tensor.wait_ge(dma_sem, slab_ready(0))
                            tensor.wait_ge(dma2_sem, slab2_ready(0))
                        if k >= 4:
                            tensor.wait_ge(red_sem, k - 3)
                        # 3 fp8 DoubleRow matmuls (e[0:256), tap via offset)
                        for w in range(W):
                            cl = g * GN + w
                            tensor.matmul(
                                cps[k % 4][:, :],
                                Wc8[:, 3 * n + w, :, :],
                                X8[:, s % SLOTS, n, :, cl : cl + GN],
                                start=(w == 0),
                                stop=False,
                                perf_mode=mybir.MatmulPerfMode.DoubleRow,
                            )
                        # 3 fp16 leftover matmuls (e[256:300), shared plane)
                        for w in range(W):
                            col = (3 * n + w) * C
                            mm = tensor.matmul(
                                cps[k % 4][:, :],
                                Wc16[:, col : col + C],
                                X2[:, s % SLOTS, g * GN + w : g * GN + w + GN],
                                start=False,
                                stop=(w == 2),
                            )
                        mm.then_inc(pe_sem, 1)
                        k += 1
            # FC + dot tail
            tensor.wait_ge(act_sem, 1)
            tensor.matmul(fps[0][:, :], Wf[:, 0:O], Hr[:, 0, :], start=True, stop=True)
            tensor.matmul(
                fps[1][:, :], Wf[:, O : 2 * O], Hr[:, 1, :], start=True, stop=True
            ).then_inc(fc_sem, 1)
            tensor.wait_ge(red_sem, 2 * NG * NSLAB + 1)
            tensor.matmul(dps[:, :], ones[:, :], P[:, :], start=True, stop=True).then_inc(
                fc_sem, 1
            )

        @block.vector
        def _(vector):
            vector.memset(ones[:, :], 1.0)
            k = 0
            for s in range(NSLAB):
                for g in range(NG):
                    p = s * NG + g  # batch pair index
                    for n in range(2):
                        vector.wait_ge(pe_sem, k + 1)
                        vector.reduce_max(
                            H[:, n, 2 * p : 2 * p + 1],
                            cps[k % 4][:, 0:VALID],
                            axis=mybir.AxisListType.X,
                        )
                        vector.reduce_max(
                            H[:, n, 2 * p + 1 : 2 * p + 2],
                            cps[k % 4][:, 200 : 200 + VALID],
                            axis=mybir.AxisListType.X,
                        ).then_inc(red_sem, 1)
                        k += 1
            vector.wait_ge(act_sem, 2)
            vector.tensor_mul(P[:, :], Ofc[:, 0, :], Ofc[:, 1, :]).then_inc(red_sem, 1)

        @block.scalar
        def _(scalar):
            # second DGE queue: fp16 leftover plane (+ boot slices)
            scalar.dma_start(out=X2[:, 0, 0:402], in_=xp2[:, 0:402]).then_inc(dma2_sem, 16)
            scalar.dma_start(out=Wc16[:, :], in_=wst16[:, :]).then_inc(dma2_sem, 16)
            for s in range(NSLAB):
                if s >= SLOTS:
                    scalar.wait_ge(pe_sem, 2 * NG * (s - SLOTS + 1))
                scalar.dma_start(
                    out=X2[:, s % SLOTS, :],
                    in_=xp2[:, s * SLAB : s * SLAB + SLABW],
                ).then_inc(dma2_sem, 16)
            scalar.wait_ge(red_sem, 2 * NG * NSLAB)
            scalar.wait_ge(dma_sem, 160)
            scalar.activation(
                Hr[:, 0, :], H[:, 0, :], mybir.ActivationFunctionType.Relu,
                bias=Bc[:, 0:1], scale=1.0 / WSCALE,
            )
            scalar.activation(
                Hr[:, 1, :], H[:, 1, :], mybir.ActivationFunctionType.Relu,
                bias=Bc[:, 1:2], scale=1.0 / WSCALE,
            ).then_inc(act_sem, 1)
            scalar.wait_ge(fc_sem, 1)
            scalar.activation(
                Ofc[:, 0, :], fps[0][:, :], mybir.ActivationFunctionType.Relu,
                bias=Bf[:, 0:1],
            )
            scalar.activation(
                Ofc[:, 1, :], fps[1][:, :], mybir.ActivationFunctionType.Relu,
                bias=Bf[:, 1:2],
            ).then_inc(act_sem, 1)
            scalar.wait_ge(fc_sem, 2)
            scalar.copy(osb[:, :], dps[:, :]).then_inc(act_sem, 1)

    return nc


def _prep_weights(w_conv1, w_conv2, w_fc1, w_fc2, b_conv1, b_conv2, b_fc1, b_fc2):
    wst8 = np.zeros((128, 6, 2, C), dtype=np.float32)
    wst16 = np.zeros((88, 6, C), dtype=np.float32)
    for n, wc in enumerate((w_conv1, w_conv2)):
        wcs = wc[:, 0] * WSCALE  # [C, W, E]
        for w in range(W):
            blk = 3 * n + w
            wst8[:, blk, 0, :] = wcs[:, w, 0:128].T
            wst8[:, blk, 1, :] = wcs[:, w, 128:256].T
            if n == 0:
                wst16[0:44, blk, :] = wcs[:, w, 256:300].T
            else:
                wst16[44:88, blk, :] = wcs[:, w, 256:300].T
    wf = np.concatenate([w_fc1.T, w_fc2.T], axis=1).astype(np.float32)  # [C, 2O]
    bc = np.stack([b_conv1, b_conv2], axis=1).astype(np.float32)  # [C, 2]
    bf = np.stack([b_fc1, b_fc2], axis=1).astype(np.float32)  # [O, 2]
    return (
        wst8.astype(NP_F8),
        wst16.reshape(88, 6 * C).astype(np.float16),
        wf, bc, bf,
    )


def kernel(x1, x2, w_conv1, b_conv1, w_fc1, b_fc1, w_conv2, b_conv2, w_fc2, b_fc2):
    global LAST_RESULT
    x1 = np.asarray(x1, dtype=np.float32)
    x2 = np.asarray(x2, dtype=np.float32)
    wst8, wst16, wf, bc, bf = _prep_weights(
        np.asarray(w_conv1, np.float32), np.asarray(w_conv2, np.float32),
        np.asarray(w_fc1, np.float32), np.asarray(w_fc2, np.float32),
        np.asarray(b_conv1, np.float32), np.asarray(b_conv2, np.float32),
        np.asarray(b_fc1, np.float32), np.asarray(b_fc2, np.float32),
    )

    if "nc" not in _NC_CACHE:
        _NC_CACHE["nc"] = _build_nc()
    nc = _NC_CACHE["nc"]

    in_maps = []
    for c in range(NCORES):
        xs1 = x1[c * BS : (c + 1) * BS, 0].reshape(POS, E).T  # [300, POS] f32
        xs2 = x2[c * BS : (c + 1) * BS, 0].reshape(POS, E).T
        xq8 = np.zeros((128, 2, 2, PADPOS), dtype=NP_F8)
        for n, xs in enumerate((xs1, xs2)):
            xq8[:, n, 0, :POS] = xs[0:128].astype(NP_F8)
            xq8[:, n, 1, :POS] = xs[128:256].astype(NP_F8)
        xp2 = np.zeros((88, PADPOS), dtype=np.float16)
        xp2[0:44, :POS] = xs1[256:300]
        xp2[44:88, :POS] = xs2[256:300]
        in_maps.append(
            {"xq8": xq8, "xp2": xp2, "wst8": wst8, "wst16": wst16,
             "wf": wf, "bc": bc, "bf": bf}
        )

    res = run_bass_kernel_spmd(nc, in_maps, list(range(NCORES)), trace=TRACE)
    LAST_RESULT = res
    return np.concatenate(
        [res.results[c]["out"].reshape(BS, 1) for c in range(NCORES)], axis=0
    )


# revision 39
# speedup vs baseline: 1.9953x; 1.0181x over previous
"""JoinConvNet Trainium2 kernel — 8-core data-parallel, raw Bass.

Math per subnet (reference):
  conv(x, w)  : x[B,1,L,E], w[C,1,W,E], VALID -> c[B,C,L-W+1]
  m = max_l c ; h = relu(m + b_conv) ; o = relu(h @ w_fc.T + b_fc)
  out[b] = dot(o1[b], o2[b])

Device strategy (per core, 128 batches):
  The e[0:256) part of the contraction runs in fp8e4m3 with
  MatmulPerfMode.DoubleRow: host packs pairs (x[e], x[e+128]) interleaved
  along the moving free dim and weights as [K,2,M] pair tiles, so one
  DoubleRow matmul per tap contracts 256 e-values at 0.5 cycles/row.
  The e[256:300) leftover runs in fp16 via the shared 88-row plane
  (x1 rows 0:44, x2 rows 44:88, zero-padded stationary) with the tap
  shift applied through the moving-column offset.

  Per (group of 400 positions, subnet): 3 DoubleRow + 3 fp16 accumulating
  matmuls -> PSUM [80,400] -> per-batch reduce_max -> H[80,128] ->
  bias+relu -> FC matmul -> bias+relu -> elementwise mul -> ones-matmul
  partition sum -> [1,128] out.

  Startup is bootstrapped with small leading DMAs so the PE starts early.
  fp8 end-to-end rel err vs the fp32 reference: ~1.5e-2 (measured, fixed
  inputs), under the 2e-2 gate.
"""
import os
import numpy as np
import ml_dtypes
from contextlib import ExitStack

import concourse.bass as bass
import concourse.mybir as mybir
from concourse.bass_utils import run_bass_kernel_spmd

B, L, E = 1024, 200, 300
C, W, O = 80, 3, 30
NCORES = 8
BS = B // NCORES            # 128 batches/core
POS = BS * L                # 25600 positions/core
PADPOS = POS + 2
NSLAB = int(os.environ.get("K_NSLAB", "32"))
SLAB = POS // NSLAB         # positions per slab
SLABW = SLAB + 2            # loaded columns per slab
NG = SLAB // 400            # 400-position groups per slab
SLOTS = int(os.environ.get("K_SLOTS", "4"))
GN = 400                    # matmul moving size
VALID = L - W + 1           # 198

F8 = mybir.dt.float8e4
F16 = mybir.dt.float16
F32 = mybir.dt.float32
NP_F8 = ml_dtypes.float8_e4m3

LAST_RESULT = None
TRACE = bool(os.environ.get("KERNEL_TRACE"))
_NC_CACHE = {}


SLABW8 = (SLABW + 15) // 16 * 16  # pair-plane stride must be 16B-aligned
WSCALE = 64.0  # lift conv weights (~0.02) out of the e4m3 subnormal range


def _build_nc():
    nc = bass.Bass()
    xq8 = nc.declare_dram_parameter("xq8", [128, 2, 2, PADPOS], F8, isOutput=False)
    xp2 = nc.declare_dram_parameter("xp2", [88, PADPOS], F16, isOutput=False)
    wst8 = nc.declare_dram_parameter("wst8", [128, 6, 2, C], F8, isOutput=False)
    wst16 = nc.declare_dram_parameter("wst16", [88, 6 * C], F16, isOutput=False)
    wf = nc.declare_dram_parameter("wf", [C, 2 * O], F32, isOutput=False)
    bc = nc.declare_dram_parameter("bc", [C, 2], F32, isOutput=False)
    bf = nc.declare_dram_parameter("bf", [O, 2], F32, isOutput=False)
    out = nc.declare_dram_parameter("out", [1, BS], F32, isOutput=True)

    with ExitStack() as ctx:
        X8 = ctx.enter_context(nc.sbuf_tensor([128, SLOTS, 2, 2, SLABW8], F8))
        X2 = ctx.enter_context(nc.sbuf_tensor([88, SLOTS, SLABW], F16))
        Wc8 = ctx.enter_context(nc.sbuf_tensor([128, 6, 2, C], F8))
        Wc16 = ctx.enter_context(nc.sbuf_tensor([88, 6 * C], F16))
        Wf = ctx.enter_context(nc.sbuf_tensor([C, 2 * O], F32))
        Bc = ctx.enter_context(nc.sbuf_tensor([C, 2], F32))
        Bf = ctx.enter_context(nc.sbuf_tensor([O, 2], F32))
        ones = ctx.enter_context(nc.sbuf_tensor([O, 1], F32))
        H = ctx.enter_context(nc.sbuf_tensor([C, 2, BS], F32))
        Hr = ctx.enter_context(nc.sbuf_tensor([C, 2, BS], F32))
        Ofc = ctx.enter_context(nc.sbuf_tensor([O, 2, BS], F32))
        P = ctx.enter_context(nc.sbuf_tensor([O, BS], F32))
        osb = ctx.enter_context(nc.sbuf_tensor([1, BS], F32))
        cps = [ctx.enter_context(nc.psum_tensor(f"cps{i}", [C, GN], F32)) for i in range(4)]
        fps = [ctx.enter_context(nc.psum_tensor(f"fps{i}", [O, BS], F32)) for i in range(2)]
        dps = ctx.enter_context(nc.psum_tensor([1, BS], F32))

        dma_sem = ctx.enter_context(nc.semaphore("dma_sem"))
        dma2_sem = ctx.enter_context(nc.semaphore("dma2_sem"))
        pe_sem = ctx.enter_context(nc.semaphore("pe_sem"))
        red_sem = ctx.enter_context(nc.semaphore("red_sem"))
        act_sem = ctx.enter_context(nc.semaphore("act_sem"))
        fc_sem = ctx.enter_context(nc.semaphore("fc_sem"))
        block = ctx.enter_context(nc.Block())

        # sync queue: 4 boot DMAs (64), X8 slab s at 64+16(s+1); wf/bc/bf
        # (+48) interleaved after slab 2.
        def slab_ready(s):
            return 64 + 16 * (s + 1) + (48 if s >= 3 else 0)

        # scalar queue: 2 boot DMAs (32), X2 slab s at 32+16(s+1)
        def slab2_ready(s):
            return 32 + 16 * (s + 1)

        @block.sync
        def _(sync):
            # bootstrap: first-group slices so the PE starts early. Boot and
            # slab-0 main DMAs cover DISJOINT column ranges: a DMA re-writing
            # bytes the PE is reading corrupts them even with identical data.
            sync.dma_start(out=X8[:, 0, 0, :, 0:402], in_=xq8[:, 0, :, 0:402]).then_inc(dma_sem, 16)
            sync.dma_start(out=Wc8[:, 0:3, :, :], in_=wst8[:, 0:3, :, :]).then_inc(dma_sem, 16)
            sync.dma_start(out=X8[:, 0, 1, :, 0:402], in_=xq8[:, 1, :, 0:402]).then_inc(dma_sem, 16)
            sync.dma_start(out=Wc8[:, 3:6, :, :], in_=wst8[:, 3:6, :, :]).then_inc(dma_sem, 16)
            for s in range(NSLAB):
                if s == 3:
                    sync.dma_start(out=Wf[:, :], in_=wf[:, :]).then_inc(dma_sem, 16)
                    sync.dma_start(out=Bc[:, :], in_=bc[:, :]).then_inc(dma_sem, 16)
                    sync.dma_start(out=Bf[:, :], in_=bf[:, :]).then_inc(dma_sem, 16)
                if s >= SLOTS:
                    # slot s%SLOTS free once PE finished slab s-SLOTS
                    sync.wait_ge(pe_sem, 2 * NG * (s - SLOTS + 1))
                lo = 402 if s == 0 else 0
                sync.dma_start(
                    out=X8[:, s % SLOTS, :, :, lo:SLABW],
                    in_=xq8[:, :, :, s * SLAB + lo : s * SLAB + SLABW],
                ).then_inc(dma_sem, 16)
            sync.wait_ge(act_sem, 3)
            sync.dma_start(out=out[:, :], in_=osb[:, :]).then_inc(dma_sem, 16)

        @block.tensor
        def _(tensor):
            k = 0
            for s in range(NSLAB):
                if s >= 1:
                    tensor.wait_ge(dma_sem, slab_ready(s))
                    tensor.wait_ge(dma2_sem, slab2_ready(s))
                for g in range(NG):
                    for n in range(2):
                        if s == 0 and g == 0 and n == 0:
                            # all boot DMAs (disjoint from slab-0 mains)
                            tensor.wait_ge(dma_sem, 64)
                            tensor.wait_ge(dma2_sem, 32)
                        if s == 0 and g == 1 and n == 0:
                            tensor.wait_ge(dma_sem, slab_ready(0))
                            tensor.wait_ge(dma2_sem, slab2_ready(0))
                        if k >= 4:
                            tensor.wait_ge(red_sem, k - 3)
                        # 3 fp8 DoubleRow matmuls (e[0:256), tap via offset)
                        for w in range(W):
                            cl = g * GN + w
                            tensor.matmul(
                                cps[k % 4][:, :],
                                Wc8[:, 3 * n + w, :, :],
                                X8[:, s % SLOTS, n, :, cl : cl + GN],
                                start=(w == 0),
                                stop=False,
                                perf_mode=mybir.MatmulPerfMode.DoubleRow,
                            )
                        # 3 fp16 leftover matmuls (e[256:300), shared plane)
                        for w in range(W):
                            col = (3 * n + w) * C
                            mm = tensor.matmul(
                                cps[k % 4][:, :],
                                Wc16[:, col : col + C],
                                X2[:, s % SLOTS, g * GN + w : g * GN + w + GN],
                                start=False,
                                stop=(w == 2),
                            )
                        mm.then_inc(pe_sem, 1)
                        k += 1
            # FC + dot tail
            tensor.wait_ge(act_sem, 1)
            tensor.matmul(fps[0][:, :], Wf[:, 0:O], Hr[:, 0, :], start=True, stop=True)
            tensor.matmul(
                fps[1][:, :], Wf[:, O : 2 * O], Hr[:, 1, :], start=True, stop=True
            ).then_inc(fc_sem, 1)
            tensor.wait_ge(red_sem, 2 * NG * NSLAB + 1)
            tensor.matmul(dps[:, :], ones[:, :], P[:, :], start=True, stop=True).then_inc(
                fc_sem, 1
            )

        @block.vector
        def _(vector):
            vector.memset(ones[:, :], 1.0)
            k = 0
            for s in range(NSLAB):
                for g in range(NG):
                    p = s * NG + g  # batch pair index
                    for n in range(2):
                        vector.wait_ge(pe_sem, k + 1)
                        vector.reduce_max(
                            H[:, n, 2 * p : 2 * p + 1],
                            cps[k % 4][:, 0:VALID],
                            axis=mybir.AxisListType.X,
                        )
                        vector.reduce_max(
                            H[:, n, 2 * p + 1 : 2 * p + 2],
                            cps[k % 4][:, 200 : 200 + VALID],
                            axis=mybir.AxisListType.X,
                        ).then_inc(red_sem, 1)
                        k += 1
            vector.wait_ge(act_sem, 2)
            vector.tensor_mul(P[:, :], Ofc[:, 0, :], Ofc[:, 1, :]).then_inc(red_sem, 1)

        @block.scalar
        def _(scalar):
            # second DGE queue: fp16 leftover plane (+ disjoint boot slices)
            scalar.dma_start(out=X2[:, 0, 0:402], in_=xp2[:, 0:402]).then_inc(dma2_sem, 16)
            scalar.dma_start(out=Wc16[:, :], in_=wst16[:, :]).then_inc(dma2_sem, 16)
            for s in range(NSLAB):
                if s >= SLOTS:
                    scalar.wait_ge(pe_sem, 2 * NG * (s - SLOTS + 1))
                lo = 402 if s == 0 else 0
                scalar.dma_start(
                    out=X2[:, s % SLOTS, lo:SLABW],
                    in_=xp2[:, s * SLAB + lo : s * SLAB + SLABW],
                ).then_inc(dma2_sem, 16)
            scalar.wait_ge(red_sem, 2 * NG * NSLAB)
            scalar.wait_ge(dma_sem, 160)
            scalar.activation(
                Hr[:, 0, :], H[:, 0, :], mybir.ActivationFunctionType.Relu,
                bias=Bc[:, 0:1], scale=1.0 / WSCALE,
            )
            scalar.activation(
                Hr[:, 1, :], H[:, 1, :], mybir.ActivationFunctionType.Relu,
                bias=Bc[:, 1:2], scale=1.0 / WSCALE,
            ).then_inc(act_sem, 1)
            scalar.wait_ge(fc_sem, 1)
            scalar.activation(
                Ofc[:, 0, :], fps[0][:, :], mybir.ActivationFunctionType.Relu,
                bias=Bf[:, 0:1],
            )
            scalar.activation(
                Ofc[:, 1, :], fps[1][:, :], mybir.ActivationFunctionType.Relu,
                bias=Bf[:, 1:2],
            ).then_inc(act_sem, 1)
            scalar.wait_ge(fc_sem, 2)
            scalar.copy(osb[:, :], dps[:, :]).then_inc(act_sem, 1)

    return nc


def _prep_weights(w_conv1, w_conv2, w_fc1, w_fc2, b_conv1, b_conv2, b_fc1, b_fc2):
    wst8 = np.zeros((128, 6, 2, C), dtype=np.float32)
    wst16 = np.zeros((88, 6, C), dtype=np.float32)
    for n, wc in enumerate((w_conv1, w_conv2)):
        wcs = wc[:, 0] * WSCALE  # [C, W, E]
        for w in range(W):
            blk = 3 * n + w
            wst8[:, blk, 0, :] = wcs[:, w, 0:128].T
            wst8[:, blk, 1, :] = wcs[:, w, 128:256].T
            if n == 0:
                wst16[0:44, blk, :] = wcs[:, w, 256:300].T
            else:
                wst16[44:88, blk, :] = wcs[:, w, 256:300].T
    wf = np.concatenate([w_fc1.T, w_fc2.T], axis=1).astype(np.float32)  # [C, 2O]
    bc = np.stack([b_conv1, b_conv2], axis=1).astype(np.float32)  # [C, 2]
    bf = np.stack([b_fc1, b_fc2], axis=1).astype(np.float32)  # [O, 2]
    return (
        wst8.astype(NP_F8),
        wst16.reshape(88, 6 * C).astype(np.float16),
        wf, bc, bf,
    )


def kernel(x1, x2, w_conv1, b_conv1, w_fc1, b_fc1, w_conv2, b_conv2, w_fc2, b_fc2):
    global LAST_RESULT
    x1 = np.asarray(x1, dtype=np.float32)
    x2 = np.asarray(x2, dtype=np.float32)
    wst8, wst16, wf, bc, bf = _prep_weights(
        np.asarray(w_conv1, np.float32), np.asarray(w_conv2, np.float32),
        np.asarray(w_fc1, np.float32), np.asarray(w_fc2, np.float32),
        np.asarray(b_conv1, np.float32), np.asarray(b_conv2, np.float32),
        np.asarray(b_fc1, np.float32), np.asarray(b_fc2, np.float32),
    )

    if "nc" not in _NC_CACHE:
        _NC_CACHE["nc"] = _build_nc()
    nc = _NC_CACHE["nc"]

    in_maps = []
    for c in range(NCORES):
        xs1 = x1[c * BS : (c + 1) * BS, 0].reshape(POS, E).T  # [300, POS] f32
        xs2 = x2[c * BS : (c + 1) * BS, 0].reshape(POS, E).T
        xq8 = np.zeros((128, 2, 2, PADPOS), dtype=NP_F8)
        for n, xs in enumerate((xs1, xs2)):
            xq8[:, n, 0, :POS] = xs[0:128].astype(NP_F8)
            xq8[:, n, 1, :POS] = xs[128:256].astype(NP_F8)
        xp2 = np.zeros((88, PADPOS), dtype=np.float16)
        xp2[0:44, :POS] = xs1[256:300]
        xp2[44:88, :POS] = xs2[256:300]
        in_maps.append(
            {"xq8": xq8, "xp2": xp2, "wst8": wst8, "wst16": wst16,
             "wf": wf, "bc": bc, "bf": bf}
        )

    res = run_bass_kernel_spmd(nc, in_maps, list(range(NCORES)), trace=TRACE)
    LAST_RESULT = res
    return np.concatenate(
        [res.results[c]["out"].reshape(BS, 1) for c in range(NCORES)], axis=0
    )
